# revision 11
# baseline (speedup 1.0000x reference)
"""MoE layer (B=2,S=1024,H=2048,F=5504,E=8,top-2) on 8 NeuronCores.

Strategy: balanced expert x f-chunk parallelism. Host computes the router +
top-2 dispatch. The unit of work is (expert, 128-wide f-chunk): 43 chunks x
8 experts = 344 units. Every core gets 5 f-chunks of EVERY expert
(8 "regular" position groups, identically shaped across cores because the
positions are sorted by token count), plus 3 single-chunk "extra" positions
that distribute the remaining 3 chunks per expert (43 = 5*8 + 3), banded by
token count so padding is small. All cores therefore run one identical
program whose per-core data (which chunk's weights, which expert's tokens)
differs, and every core does ~T*K/8 token-chunk work: near-perfect balance.

Each group: gate/up matmuls (PSUM fp32, bf16 in), SwiGLU on scalar+vector
engines, then a token-proportional down-proj (moving operand = tokens, out
= [h-tile, tokens]) accumulated over the group's f-chunks in PSUM. Partial
outputs stream back in bf16; the host sums partials, applies the combine
probabilities, and scatter-adds into the full [T, H] output.
"""

import sys

import numpy as np
import ml_dtypes

if "/opt/trn_rl_repo" not in sys.path:
    sys.path.insert(0, "/opt/trn_rl_repo")

B, S, H, F, E, TOPK = 2, 1024, 2048, 5504, 8, 2
T = B * S
P = 128
FT = F // P  # 43 f-chunks of 128
HC = H // P  # 16 h-chunks of 128
HT = H // P  # 16 h-tiles for down-proj output
NCORES = 8
REG = FT // NCORES  # 5 regular f-chunks per (core, expert)
NEXTRA = FT - REG * NCORES  # 3 extra positions
BF16 = ml_dtypes.bfloat16

_nc_cache: dict = {}

# test-harness knobs (harmless defaults for grading)
TRACE = False
LAST_RESULT = None


def _split_waits(nc):
    """Walrus on this toolchain encodes at most ONE sync wait per DMA-queue
    instruction (DIRECT2D EVENTS has a single wait slot) and refuses
    multi-wait drains. Tile emits multi-wait DMAs for slot-reuse (WAR vs
    readers + WAW vs previous fill). Fix up the scheduled BIR: for each
    queue DMA with N>1 waits, insert N-1 zero-update 2-byte scratch DMAs
    ("wait gates") on the same queue immediately before it, each carrying
    one wait — queue FIFO issue makes the semantics identical. Multi-wait
    drains are split into single-wait drain clones the same way."""
    import copy
    import concourse.mybir as mybir

    tmpl = None
    for f in nc.m.functions:
        for b in f.blocks:
            for ins in b.instructions:
                if type(ins).__name__ == "InstDMACopy" and ins.outs and "wgdst" in str(
                    ins.outs[0]
                ):
                    tmpl = ins
    assert tmpl is not None, "wait-gate template (wgdst dma) not found"

    k = 0
    for f in nc.m.functions:
        for b in f.blocks:
            newlist = []
            for ins in b.instructions:
                si = ins.sync_info
                tn = type(ins).__name__
                if (
                    tn == "InstDMACopy"
                    and ins.name != tmpl.name
                    and si is not None
                    and si.on_wait
                    and len(si.on_wait) > 1
                ):
                    waits = list(si.on_wait)
                    for w in waits[:-1]:
                        k += 1
                        upd = copy.deepcopy(list(tmpl.sync_info.on_update))
                        for u in upd:
                            u.update_value = 0
                        d = mybir.InstDMACopy(
                            name=f"I-{900000 + k}",
                            engine=ins.engine,
                            ins=copy.deepcopy(tmpl.ins),
                            outs=copy.deepcopy(tmpl.outs),
                            queue=getattr(ins, "queue", None) or tmpl.queue,
                            mode=tmpl.mode,
                            oob_is_err=tmpl.oob_is_err,
                            cce_op=tmpl.cce_op,
                            single_packet=tmpl.single_packet,
                            sync_info=mybir.SyncInfo(on_wait=[w], on_update=upd),
                        )
                        newlist.append(d)
                    ins.sync_info = mybir.SyncInfo(
                        on_wait=[waits[-1]], on_update=list(si.on_update or [])
                    )
                elif si is not None and si.on_wait and len(si.on_wait) > 1:
                    waits = list(si.on_wait)
                    for w in waits[:-1]:
                        k += 1
                        d = mybir.InstEventSemaphore(
                            name=f"I-{900000 + k}",
                            engine=ins.engine,
                            sync_info=mybir.SyncInfo(on_wait=[w], on_update=[]),
                        )
                        newlist.append(d)
                    ins.sync_info = mybir.SyncInfo(
                        on_wait=[waits[-1]], on_update=list(si.on_update or [])
                    )
                newlist.append(ins)
            b.instructions[:] = newlist
    return k


def _ttiles(nt: int):
    """Split nt tokens into near-equal tiles of <=512 (PSUM bank limit)."""
    n_tiles = -(-nt // 512)
    base = nt // n_tiles
    rem = nt - base * n_tiles
    out = []
    off = 0
    for i in range(n_tiles):
        n = base + (1 if i < rem else 0)
        out.append((off, n))
        off += n
    return out


def _build_nc(groups):
    """groups: tuple of (nf, nt) per position, in emission order."""
    import concourse.bass as bass
    import concourse.mybir as mybir
    from concourse.tile import TileContext
    from contextlib import ExitStack

    dt = mybir.dt
    NG = len(groups)

    nc = bass.Bass()
    xts, gts, dts, yts = [], [], [], []
    for g, (nf, nt) in enumerate(groups):
        xts.append(nc.dram_tensor(f"x{g}", [P, HC, nt], dt.bfloat16, kind="ExternalInput"))
        gts.append(nc.dram_tensor(f"g{g}", [nf, P, 2, HC * P], dt.bfloat16, kind="ExternalInput"))
        dts.append(nc.dram_tensor(f"d{g}", [P, nf, HT * P], dt.bfloat16, kind="ExternalInput"))
        yts.append(nc.dram_tensor(f"y{g}", [P, HT, nt], dt.bfloat16, kind="ExternalOutput"))
    wgsrc = nc.dram_tensor("wgsrc", [1, 1], dt.bfloat16, kind="ExternalInput")
    wgdst = nc.dram_tensor("wgdst", [1, 1], dt.bfloat16)

    with TileContext(nc) as tc, ExitStack() as ctx:
        xpool = ctx.enter_context(tc.tile_pool(name="xp", bufs=3))
        gpool = ctx.enter_context(tc.tile_pool(name="gw", bufs=6))
        dpool = ctx.enter_context(tc.tile_pool(name="dw", bufs=2))
        hpool = ctx.enter_context(tc.tile_pool(name="hp", bufs=2))
        ypool = ctx.enter_context(tc.tile_pool(name="yp", bufs=2))
        spool = ctx.enter_context(tc.tile_pool(name="st", bufs=2))
        pg = ctx.enter_context(tc.tile_pool(name="pg", bufs=2, space="PSUM"))
        pu = ctx.enter_context(tc.tile_pool(name="pu", bufs=2, space="PSUM"))
        pdn = ctx.enter_context(tc.tile_pool(name="pd", bufs=4, space="PSUM"))

        # per-group live state: x tile, h tile, d tiles, y tile
        st_x, st_h, st_d, st_y = {}, {}, {}, {}
        # cold-start gating: the DGE launches ungated DMAs immediately, so
        # first-wave transfers (no pool-WAR predecessor yet) would saturate
        # the wire during startup. Chain them with 1-elem gpsimd copies.
        chain = {"gu": None}

        def load_x(k):
            nf, nt = groups[k]
            x_sb = xpool.tile([P, HC, nt], dt.bfloat16, tag="x")
            if k == 0:
                # split the startup-critical first x across both DMA queues,
                # balancing wire time (sync also carries the first gate tile)
                nc.sync.dma_start(x_sb[:, :7], xts[k][:, :7])
                nc.scalar.dma_start(x_sb[:, 7:], xts[k][:, 7:])
            else:
                if k == 1:
                    nc.gpsimd.tensor_copy(x_sb[0:1, 0:1, 0:1], chain["gu"][0:1, 0:1, 0:1])
                elif k == 2:
                    nc.gpsimd.tensor_copy(x_sb[0:1, 0:1, 0:1], st_x[1][0:1, 0:1, 0:1])
                nc.scalar.dma_start(x_sb[:], xts[k][:])
            st_x[k] = x_sb

        def load_d(k):
            nf, nt = groups[k]
            d_sb = dpool.tile([P, nf, HT * P], dt.bfloat16, tag="d", name="dsb")
            if k == 0:
                # keep the big d0 transfer off the wire until group-0 weight
                # prefetch is done: 1-elem copy gates the DMA behind unit-1 h
                nc.vector.tensor_copy(d_sb[0:1, 0:1, 0:1], st_h[0][0:1, 0:1, 0:1])
            elif k == 1:
                nc.gpsimd.tensor_copy(d_sb[0:1, 0:1, 0:1], st_d[0][0:1, 0:1, 0:1])
            nc.scalar.dma_start(d_sb[:], dts[k][:])
            st_d[k] = d_sb

        def emit_gu_unit(k, i):
            """gate/up matmuls + SwiGLU for f-chunk i of group k."""
            nf, nt = groups[k]
            x_sb = st_x[k]
            h_sb = st_h[k]
            gu_sl = gpool.tile([P, 2, HC * P], dt.bfloat16, tag="gu", name="gusl")
            if k == 0 and i == 0:
                nc.sync.dma_start(gu_sl[:, 0], gts[k][i, :, 0])
                nc.sync.dma_start(gu_sl[:, 1], gts[k][i, :, 1])
            else:
                if (k == 0 and i >= 2) or (k == 1 and i == 0):
                    nc.gpsimd.tensor_copy(
                        gu_sl[0:1, 0:1, 0:1], chain["gu"][0:1, 0:1, 0:1]
                    )
                nc.sync.dma_start(gu_sl[:], gts[k][i])
            chain["gu"] = gu_sl
            g_sl = gu_sl[:, 0]
            u_sl = gu_sl[:, 1]

            tt = _ttiles(nt)
            psg, psu = {}, {}
            for t0, n in tt:
                gp = pg.tile([P, 512], dt.float32, tag="gp")
                for hc in range(HC):
                    nc.tensor.matmul(
                        gp[:, :n],
                        g_sl[:, hc * P : (hc + 1) * P],
                        x_sb[:, hc, t0 : t0 + n],
                        start=(hc == 0),
                        stop=(hc == HC - 1),
                    )
                psg[t0] = gp
            for t0, n in tt:
                up = pu.tile([P, 512], dt.float32, tag="up")
                for hc in range(HC):
                    nc.tensor.matmul(
                        up[:, :n],
                        u_sl[:, hc * P : (hc + 1) * P],
                        x_sb[:, hc, t0 : t0 + n],
                        start=(hc == 0),
                        stop=(hc == HC - 1),
                    )
                psu[t0] = up
            for t0, n in tt:
                st = spool.tile([P, 512], dt.float32, tag="st")
                nc.scalar.activation(
                    st[:, :n], psg[t0][:, :n], mybir.ActivationFunctionType.Silu
                )
                nc.vector.tensor_mul(
                    out=h_sb[:, i, t0 : t0 + n], in0=st[:, :n], in1=psu[t0][:, :n]
                )

        def emit_down_chunk(k, hts):
            """down-proj h-tiles `hts` of group k, PSUM-accumulated over its
            f-chunks; results stream to y (bf16) and out via the vector queue."""
            nf, nt = groups[k]
            h_sb = st_h[k]
            y_sb = st_y[k]
            d_sb = st_d[k]
            tt = _ttiles(nt)
            pend = []
            for ht in hts:
                for t0, n in tt:
                    ps = pdn.tile([P, 512], dt.float32, tag="dn")
                    for i in range(nf):
                        nc.tensor.matmul(
                            ps[:, :n],
                            d_sb[:, i, ht * P : (ht + 1) * P],
                            h_sb[:, i, t0 : t0 + n],
                            start=(i == 0),
                            stop=(i == nf - 1),
                        )
                    if ht % 2 == 0:
                        nc.scalar.activation(
                            y_sb[:, ht, t0 : t0 + n],
                            ps[:, :n],
                            mybir.ActivationFunctionType.Copy,
                        )
                    else:
                        nc.vector.tensor_copy(y_sb[:, ht, t0 : t0 + n], ps[:, :n])
                pend.append(ht)
                if len(pend) == 4:
                    eng = nc.sync if (pend[0] // 4) % 2 == 0 else nc.scalar
                    eng.dma_start(
                        yts[k][:, pend[0] : pend[-1] + 1, :],
                        y_sb[:, pend[0] : pend[-1] + 1, :],
                    )
                    pend = []
            if pend:
                eng = nc.sync if (pend[0] // 4) % 2 == 0 else nc.scalar
                eng.dma_start(
                    yts[k][:, pend[0] : pend[-1] + 1, :],
                    y_sb[:, pend[0] : pend[-1] + 1, :],
                )

        # h-tile emission schedule: down-proj of group k is interleaved into
        # the gate/up unit loop of group k+1 so the PE never waits on SwiGLU.
        def down_sched(nf_next):
            parts = [[] for _ in range(nf_next)]
            for ht in range(HT):
                parts[ht * nf_next // HT].append(ht)
            return parts

        load_x(0)
        for k, (nf, nt) in enumerate(groups):
            st_h[k] = hpool.tile([P, nf, nt], dt.bfloat16, tag="h", name="hsb")
            st_y[k] = ypool.tile([P, HT, nt], dt.bfloat16, tag="y", name="ysb")
            if k > 0:
                load_d(k)
                if k + 1 < NG:
                    load_x(k + 1)
            sched = down_sched(nf) if k > 0 else None
            for i in range(nf):
                emit_gu_unit(k, i)
                if k == 0 and i == 1:
                    load_d(k)
                if k == 0 and i == nf - 1:
                    load_x(k + 1)
                if k > 0:
                    emit_down_chunk(k - 1, sched[i])
                    if i == nf - 1:
                        # previous group fully consumed: drop references
                        st_x.pop(k - 1, None)
                        st_h.pop(k - 1, None)
                        st_d.pop(k - 1, None)
                        st_y.pop(k - 1, None)
        emit_down_chunk(NG - 1, list(range(HT)))

        # template for the wait-gate post-pass (see _split_waits)
        nc.sync.dma_start(wgdst[:], wgsrc[:])

    _split_waits(nc)
    return nc


def _route(xf: np.ndarray, router_w: np.ndarray):
    """Top-2 routing, reproducing jax.lax.top_k (ties -> lower index) and
    softmax over the two selected logits."""
    logits = xf.astype(np.float64) @ router_w.astype(np.float64).T  # [T, E]
    order = np.argsort(-logits, axis=-1, kind="stable")[:, :TOPK]  # [T, 2]
    top_v = np.take_along_axis(logits, order, axis=1)
    ex = np.exp(top_v - top_v.max(axis=1, keepdims=True))
    probs = (ex / ex.sum(axis=1, keepdims=True)).astype(np.float32)
    return order, probs


def _schedule(counts):
    """Build the uniform program table and per-core unit assignment.

    Returns (groups, assign) where groups[g] = (nf, nt_padded) and
    assign[c][g] = (expert, [f-chunk indices]).
    """
    ranks = sorted(range(E), key=lambda e: -counts[e])
    groups = []
    assign = [[] for _ in range(NCORES)]
    for k in range(NCORES):
        e = ranks[k]
        groups.append((REG, max(int(counts[e]), 1)))
        for c in range(NCORES):
            assign[c].append((e, list(range(c * REG, (c + 1) * REG))))
    # extras: each expert's f-chunks [REG*NCORES, FT) dealt across cores,
    # banded by token count to minimize padding
    deal = [e for e in ranks for _ in range(NEXTRA)]  # sorted desc by count
    extra_ptr = {e: REG * NCORES for e in range(E)}
    for j in range(NEXTRA):
        band = deal[j * NCORES : (j + 1) * NCORES]
        nt = max(max(int(counts[e]) for e in band), 1)
        groups.append((1, nt))
        for c in range(NCORES):
            e = band[c]
            assign[c].append((e, [extra_ptr[e]]))
            extra_ptr[e] += 1
    assert all(extra_ptr[e] == FT for e in range(E))
    # emission order: regular groups ascending by nt (small first => fast
    # start) with the extras interleaved mid-stream so their DMA bursts
    # amortize into the big groups' windows (extras are single-chunk and
    # locally DMA-bound); the final group is a regular 5-chunk one whose
    # down-proj tail overlaps the output drain
    regs = sorted(range(NCORES), key=lambda g: groups[g][1])
    exts = sorted(range(NCORES, NCORES + NEXTRA), key=lambda g: groups[g][1], reverse=True)
    order = [regs[0], regs[1], exts[0], regs[2], regs[3], exts[1], regs[4], regs[5], exts[2], regs[6], regs[7]]
    groups = [groups[g] for g in order]
    assign = [[asg[g] for g in order] for asg in assign]
    return groups, assign


def _prep_weight_unit(gate_w, up_w, down_w, e, fc):
    """lhsT layouts for one (expert, f-chunk) unit."""
    g16 = gate_w[e][fc * P : (fc + 1) * P].astype(BF16)  # [128, H]
    u16 = up_w[e][fc * P : (fc + 1) * P].astype(BF16)  # [128, H]
    d16 = down_w[e][:, fc * P : (fc + 1) * P].astype(BF16)  # [H, 128]
    # gate/up lhsT: [hp, hc*128+fi] = w[fi, hc*128+hp]
    gt = np.ascontiguousarray(g16.reshape(P, HC, P).transpose(2, 1, 0)).reshape(
        P, HC * P
    )
    ut = np.ascontiguousarray(u16.reshape(P, HC, P).transpose(2, 1, 0)).reshape(
        P, HC * P
    )
    # down lhsT: [fp, ht*128+hi] = d[ht*128+hi, fp]
    dtt = np.ascontiguousarray(d16.T)  # [128, H]
    return gt, ut, dtt


def kernel(x, router_w, gate_w, up_w, down_w):
    from concourse.bass_utils import run_bass_kernel_spmd

    x = np.asarray(x)
    router_w = np.asarray(router_w)
    gate_w = np.asarray(gate_w)
    up_w = np.asarray(up_w)
    down_w = np.asarray(down_w)

    xf = x.reshape(T, H)
    order, probs = _route(xf, router_w)

    # per-expert token lists + combine weights
    idxs, pes = [], []
    for e in range(E):
        sel = (order[:, 0] == e) | (order[:, 1] == e)
        idx = np.nonzero(sel)[0]
        pe = np.where(order[idx, 0] == e, probs[idx, 0], probs[idx, 1])
        idxs.append(idx)
        pes.append(pe.astype(np.float32))
    counts = [len(i) for i in idxs]

    groups, assign = _schedule(counts)
    key = tuple(groups)
    if key not in _nc_cache:
        _nc_cache[key] = _build_nc(groups)
    nc = _nc_cache[key]

    # gathered x per expert in [hp, hc, t] layout (shared across cores)
    xg_cache = {}

    def xg_for(e, nt):
        kk = (e, nt)
        if kk not in xg_cache:
            pad = np.zeros((nt, H), dtype=BF16)
            pad[: counts[e]] = xf[idxs[e]].astype(BF16)
            xg_cache[kk] = np.ascontiguousarray(
                pad.reshape(nt, HC, P).transpose(2, 1, 0)
            )
        return xg_cache[kk]

    in_maps = []
    for c in range(NCORES):
        m = {"wgsrc": np.zeros((1, 1), dtype=BF16)}
        for g, (nf, nt) in enumerate(groups):
            e, fcs = assign[c][g]
            m[f"x{g}"] = xg_for(e, nt)
            gustack = np.empty((nf, P, 2, HC * P), dtype=BF16)
            dstack = np.empty((P, nf, HT * P), dtype=BF16)
            for i, fc in enumerate(fcs):
                gt, ut, dtt = _prep_weight_unit(gate_w, up_w, down_w, e, fc)
                gustack[i, :, 0] = gt
                gustack[i, :, 1] = ut
                dstack[:, i] = dtt
            m[f"g{g}"] = gustack
            m[f"d{g}"] = dstack
        in_maps.append(m)

    res = run_bass_kernel_spmd(
        nc, in_maps, core_ids=list(range(NCORES)), trace=TRACE
    )
    global LAST_RESULT
    LAST_RESULT = res

    # host combine: sum bf16 partials per expert, apply probs, scatter-add
    y_acc = [np.zeros((H, counts[e]), dtype=np.float32) for e in range(E)]
    for c in range(NCORES):
        for g, (nf, nt) in enumerate(groups):
            e, _ = assign[c][g]
            part = np.asarray(res.results[c][f"y{g}"])  # [P, HT, nt] bf16
            part = part.transpose(1, 0, 2).reshape(H, nt)
            y_acc[e] += part[:, : counts[e]].astype(np.float32)
    out_flat = np.zeros((T, H), dtype=np.float32)
    for e in range(E):
        out_flat[idxs[e]] += pes[e][:, None] * y_acc[e].T
    return out_flat.reshape(B, S, H)


# revision 12
# speedup vs baseline: 1.0288x; 1.0288x over previous
"""MoE layer (B=2,S=1024,H=2048,F=5504,E=8,top-2) on 8 NeuronCores.

Strategy: balanced expert x f-chunk parallelism. Host computes the router +
top-2 dispatch. The unit of work is (expert, 128-wide f-chunk): 43 chunks x
8 experts = 344 units. Every core gets 5 f-chunks of EVERY expert
(8 "regular" position groups, identically shaped across cores because the
positions are sorted by token count), plus 3 single-chunk "extra" positions
that distribute the remaining 3 chunks per expert (43 = 5*8 + 3), banded by
token count so padding is small. All cores therefore run one identical
program whose per-core data (which chunk's weights, which expert's tokens)
differs, and every core does ~T*K/8 token-chunk work: near-perfect balance.

Each group: gate/up matmuls (PSUM fp32, bf16 in), SwiGLU on scalar+vector
engines, then a token-proportional down-proj (moving operand = tokens, out
= [h-tile, tokens]) accumulated over the group's f-chunks in PSUM. Partial
outputs stream back in bf16; the host sums partials, applies the combine
probabilities, and scatter-adds into the full [T, H] output.
"""

import sys

import numpy as np
import ml_dtypes

if "/opt/trn_rl_repo" not in sys.path:
    sys.path.insert(0, "/opt/trn_rl_repo")

B, S, H, F, E, TOPK = 2, 1024, 2048, 5504, 8, 2
T = B * S
P = 128
FT = F // P  # 43 f-chunks of 128
HC = H // P  # 16 h-chunks of 128
HT = H // P  # 16 h-tiles for down-proj output
NCORES = 8
REG = FT // NCORES  # 5 regular f-chunks per (core, expert)
NEXTRA = FT - REG * NCORES  # 3 extra positions
BF16 = ml_dtypes.bfloat16

_nc_cache: dict = {}

# test-harness knobs (harmless defaults for grading)
TRACE = False
LAST_RESULT = None


def _split_waits(nc):
    """Walrus on this toolchain encodes at most ONE sync wait per DMA-queue
    instruction (DIRECT2D EVENTS has a single wait slot) and refuses
    multi-wait drains. Tile emits multi-wait DMAs for slot-reuse (WAR vs
    readers + WAW vs previous fill). Fix up the scheduled BIR: for each
    queue DMA with N>1 waits, insert N-1 zero-update 2-byte scratch DMAs
    ("wait gates") on the same queue immediately before it, each carrying
    one wait — queue FIFO issue makes the semantics identical. Multi-wait
    drains are split into single-wait drain clones the same way."""
    import copy
    import concourse.mybir as mybir

    tmpl = None
    for f in nc.m.functions:
        for b in f.blocks:
            for ins in b.instructions:
                if type(ins).__name__ == "InstDMACopy" and ins.outs and "wgdst" in str(
                    ins.outs[0]
                ):
                    tmpl = ins
    assert tmpl is not None, "wait-gate template (wgdst dma) not found"

    k = 0
    for f in nc.m.functions:
        for b in f.blocks:
            newlist = []
            for ins in b.instructions:
                si = ins.sync_info
                tn = type(ins).__name__
                if (
                    tn == "InstDMACopy"
                    and ins.name != tmpl.name
                    and si is not None
                    and si.on_wait
                    and len(si.on_wait) > 1
                ):
                    waits = list(si.on_wait)
                    for w in waits[:-1]:
                        k += 1
                        upd = copy.deepcopy(list(tmpl.sync_info.on_update))
                        for u in upd:
                            u.update_value = 0
                        d = mybir.InstDMACopy(
                            name=f"I-{900000 + k}",
                            engine=ins.engine,
                            ins=copy.deepcopy(tmpl.ins),
                            outs=copy.deepcopy(tmpl.outs),
                            queue=getattr(ins, "queue", None) or tmpl.queue,
                            mode=tmpl.mode,
                            oob_is_err=tmpl.oob_is_err,
                            cce_op=tmpl.cce_op,
                            single_packet=tmpl.single_packet,
                            sync_info=mybir.SyncInfo(on_wait=[w], on_update=upd),
                        )
                        newlist.append(d)
                    ins.sync_info = mybir.SyncInfo(
                        on_wait=[waits[-1]], on_update=list(si.on_update or [])
                    )
                elif si is not None and si.on_wait and len(si.on_wait) > 1:
                    waits = list(si.on_wait)
                    for w in waits[:-1]:
                        k += 1
                        d = mybir.InstEventSemaphore(
                            name=f"I-{900000 + k}",
                            engine=ins.engine,
                            sync_info=mybir.SyncInfo(on_wait=[w], on_update=[]),
                        )
                        newlist.append(d)
                    ins.sync_info = mybir.SyncInfo(
                        on_wait=[waits[-1]], on_update=list(si.on_update or [])
                    )
                newlist.append(ins)
            b.instructions[:] = newlist
    return k


def _ttiles(nt: int):
    """Split nt tokens into near-equal tiles of <=512 (PSUM bank limit)."""
    n_tiles = -(-nt // 512)
    base = nt // n_tiles
    rem = nt - base * n_tiles
    out = []
    off = 0
    for i in range(n_tiles):
        n = base + (1 if i < rem else 0)
        out.append((off, n))
        off += n
    return out


def _build_nc(groups):
    """groups: tuple of (nf, nt) per position, in emission order."""
    import concourse.bass as bass
    import concourse.mybir as mybir
    from concourse.tile import TileContext
    from contextlib import ExitStack

    dt = mybir.dt
    NG = len(groups)

    nc = bass.Bass()
    xts, gts, dts, yts = [], [], [], []
    for g, (nf, nt) in enumerate(groups):
        xts.append(nc.dram_tensor(f"x{g}", [P, HC, nt], dt.bfloat16, kind="ExternalInput"))
        gts.append(nc.dram_tensor(f"g{g}", [nf, P, 2, HC * P], dt.bfloat16, kind="ExternalInput"))
        dts.append(nc.dram_tensor(f"d{g}", [P, nf, HT * P], dt.bfloat16, kind="ExternalInput"))
        yts.append(nc.dram_tensor(f"y{g}", [P, HT, nt], dt.bfloat16, kind="ExternalOutput"))
    wgsrc = nc.dram_tensor("wgsrc", [1, 1], dt.bfloat16, kind="ExternalInput")
    wgdst = nc.dram_tensor("wgdst", [1, 1], dt.bfloat16)

    with TileContext(nc) as tc, ExitStack() as ctx:
        xpool = ctx.enter_context(tc.tile_pool(name="xp", bufs=3))
        gpool = ctx.enter_context(tc.tile_pool(name="gw", bufs=6))
        dpool = ctx.enter_context(tc.tile_pool(name="dw", bufs=2))
        hpool = ctx.enter_context(tc.tile_pool(name="hp", bufs=2))
        ypool = ctx.enter_context(tc.tile_pool(name="yp", bufs=2))
        spool = ctx.enter_context(tc.tile_pool(name="st", bufs=2))
        pg = ctx.enter_context(tc.tile_pool(name="pg", bufs=2, space="PSUM"))
        pu = ctx.enter_context(tc.tile_pool(name="pu", bufs=2, space="PSUM"))
        pdn = ctx.enter_context(tc.tile_pool(name="pd", bufs=4, space="PSUM"))

        # per-group live state: x tile, h tile, d tiles, y tile
        st_x, st_h, st_d, st_y = {}, {}, {}, {}
        # cold-start gating: the DGE launches ungated DMAs immediately, so
        # first-wave transfers (no pool-WAR predecessor yet) would saturate
        # the wire during startup. Gate them all on the arrival of unit-0's
        # weights (1-elem gpsimd copies) - past the critical window, with
        # plenty of lead before each is needed.
        chain = {"root": None}

        def load_x(k):
            nf, nt = groups[k]
            x_sb = xpool.tile([P, HC, nt], dt.bfloat16, tag="x")
            if k == 0:
                # split the startup-critical first x across both DMA queues,
                # balancing wire time (sync also carries the first gate tile)
                nc.sync.dma_start(x_sb[:, :7], xts[k][:, :7])
                nc.scalar.dma_start(x_sb[:, 7:], xts[k][:, 7:])
            else:
                if k == 1:
                    nc.gpsimd.tensor_copy(x_sb[0:1, 0:1, 0:1], chain["root"][0:1, 1:2, 0:1])
                elif k == 2:
                    nc.gpsimd.tensor_copy(x_sb[0:1, 0:1, 0:1], st_x[1][0:1, 0:1, 0:1])
                nc.scalar.dma_start(x_sb[:], xts[k][:])
            st_x[k] = x_sb

        def load_d(k):
            nf, nt = groups[k]
            d_sb = dpool.tile([P, nf, HT * P], dt.bfloat16, tag="d", name="dsb")
            if k == 0:
                # keep the big d0 transfer off the wire until group-0 weight
                # prefetch is done: 1-elem copy gates the DMA behind unit-1 h
                nc.vector.tensor_copy(d_sb[0:1, 0:1, 0:1], st_h[0][0:1, 0:1, 0:1])
            elif k == 1:
                nc.gpsimd.tensor_copy(d_sb[0:1, 0:1, 0:1], st_d[0][0:1, 0:1, 0:1])
            nc.scalar.dma_start(d_sb[:], dts[k][:])
            st_d[k] = d_sb

        def emit_gu_unit(k, i):
            """gate/up matmuls + SwiGLU for f-chunk i of group k."""
            nf, nt = groups[k]
            x_sb = st_x[k]
            h_sb = st_h[k]
            gu_sl = gpool.tile([P, 2, HC * P], dt.bfloat16, tag="gu", name="gusl")
            if k == 0 and i == 0:
                nc.sync.dma_start(gu_sl[:, 0], gts[k][i, :, 0])
                nc.sync.dma_start(gu_sl[:, 1], gts[k][i, :, 1])
            else:
                if (k == 0 and i >= 2) or (k == 1 and i == 0):
                    nc.gpsimd.tensor_copy(
                        gu_sl[0:1, 0:1, 0:1], chain["root"][0:1, 1:2, 0:1]
                    )
                nc.sync.dma_start(gu_sl[:], gts[k][i])
            if k == 0 and i == 0:
                chain["root"] = gu_sl
            g_sl = gu_sl[:, 0]
            u_sl = gu_sl[:, 1]

            tt = _ttiles(nt)
            psg, psu = {}, {}
            for t0, n in tt:
                gp = pg.tile([P, 512], dt.float32, tag="gp")
                for hc in range(HC):
                    nc.tensor.matmul(
                        gp[:, :n],
                        g_sl[:, hc * P : (hc + 1) * P],
                        x_sb[:, hc, t0 : t0 + n],
                        start=(hc == 0),
                        stop=(hc == HC - 1),
                    )
                psg[t0] = gp
            for t0, n in tt:
                up = pu.tile([P, 512], dt.float32, tag="up")
                for hc in range(HC):
                    nc.tensor.matmul(
                        up[:, :n],
                        u_sl[:, hc * P : (hc + 1) * P],
                        x_sb[:, hc, t0 : t0 + n],
                        start=(hc == 0),
                        stop=(hc == HC - 1),
                    )
                psu[t0] = up
            for t0, n in tt:
                st = spool.tile([P, 512], dt.float32, tag="st")
                nc.scalar.activation(
                    st[:, :n], psg[t0][:, :n], mybir.ActivationFunctionType.Silu
                )
                nc.vector.tensor_mul(
                    out=h_sb[:, i, t0 : t0 + n], in0=st[:, :n], in1=psu[t0][:, :n]
                )

        def emit_down_chunk(k, hts):
            """down-proj h-tiles `hts` of group k, PSUM-accumulated over its
            f-chunks; results stream to y (bf16) and out via the vector queue."""
            nf, nt = groups[k]
            h_sb = st_h[k]
            y_sb = st_y[k]
            d_sb = st_d[k]
            tt = _ttiles(nt)
            pend = []
            for ht in hts:
                for t0, n in tt:
                    ps = pdn.tile([P, 512], dt.float32, tag="dn")
                    for i in range(nf):
                        nc.tensor.matmul(
                            ps[:, :n],
                            d_sb[:, i, ht * P : (ht + 1) * P],
                            h_sb[:, i, t0 : t0 + n],
                            start=(i == 0),
                            stop=(i == nf - 1),
                        )
                    if ht % 2 == 0:
                        nc.scalar.activation(
                            y_sb[:, ht, t0 : t0 + n],
                            ps[:, :n],
                            mybir.ActivationFunctionType.Copy,
                        )
                    else:
                        nc.vector.tensor_copy(y_sb[:, ht, t0 : t0 + n], ps[:, :n])
                pend.append(ht)
                if len(pend) == 4:
                    eng = nc.sync if (pend[0] // 4) % 2 == 0 else nc.scalar
                    eng.dma_start(
                        yts[k][:, pend[0] : pend[-1] + 1, :],
                        y_sb[:, pend[0] : pend[-1] + 1, :],
                    )
                    pend = []
            if pend:
                eng = nc.sync if (pend[0] // 4) % 2 == 0 else nc.scalar
                eng.dma_start(
                    yts[k][:, pend[0] : pend[-1] + 1, :],
                    y_sb[:, pend[0] : pend[-1] + 1, :],
                )

        # h-tile emission schedule: down-proj of group k is interleaved into
        # the gate/up unit loop of group k+1 so the PE never waits on SwiGLU.
        def down_sched(nf_next):
            parts = [[] for _ in range(nf_next)]
            for ht in range(HT):
                parts[ht * nf_next // HT].append(ht)
            return parts

        load_x(0)
        for k, (nf, nt) in enumerate(groups):
            st_h[k] = hpool.tile([P, nf, nt], dt.bfloat16, tag="h", name="hsb")
            st_y[k] = ypool.tile([P, HT, nt], dt.bfloat16, tag="y", name="ysb")
            if k > 0:
                load_d(k)
                if k + 1 < NG:
                    load_x(k + 1)
            sched = down_sched(nf) if k > 0 else None
            for i in range(nf):
                emit_gu_unit(k, i)
                if k == 0 and i == 1:
                    load_d(k)
                    load_x(k + 1)
                if k > 0:
                    emit_down_chunk(k - 1, sched[i])
                    if i == nf - 1:
                        # previous group fully consumed: drop references
                        st_x.pop(k - 1, None)
                        st_h.pop(k - 1, None)
                        st_d.pop(k - 1, None)
                        st_y.pop(k - 1, None)
        emit_down_chunk(NG - 1, list(range(HT)))

        # template for the wait-gate post-pass (see _split_waits)
        nc.sync.dma_start(wgdst[:], wgsrc[:])

    _split_waits(nc)
    return nc


def _route(xf: np.ndarray, router_w: np.ndarray):
    """Top-2 routing, reproducing jax.lax.top_k (ties -> lower index) and
    softmax over the two selected logits."""
    logits = xf.astype(np.float64) @ router_w.astype(np.float64).T  # [T, E]
    order = np.argsort(-logits, axis=-1, kind="stable")[:, :TOPK]  # [T, 2]
    top_v = np.take_along_axis(logits, order, axis=1)
    ex = np.exp(top_v - top_v.max(axis=1, keepdims=True))
    probs = (ex / ex.sum(axis=1, keepdims=True)).astype(np.float32)
    return order, probs


def _schedule(counts):
    """Build the uniform program table and per-core unit assignment.

    Returns (groups, assign) where groups[g] = (nf, nt_padded) and
    assign[c][g] = (expert, [f-chunk indices]).
    """
    ranks = sorted(range(E), key=lambda e: -counts[e])
    groups = []
    assign = [[] for _ in range(NCORES)]
    for k in range(NCORES):
        e = ranks[k]
        groups.append((REG, max(int(counts[e]), 1)))
        for c in range(NCORES):
            assign[c].append((e, list(range(c * REG, (c + 1) * REG))))
    # extras: each expert's f-chunks [REG*NCORES, FT) dealt across cores,
    # banded by token count to minimize padding
    deal = [e for e in ranks for _ in range(NEXTRA)]  # sorted desc by count
    extra_ptr = {e: REG * NCORES for e in range(E)}
    for j in range(NEXTRA):
        band = deal[j * NCORES : (j + 1) * NCORES]
        nt = max(max(int(counts[e]) for e in band), 1)
        groups.append((1, nt))
        for c in range(NCORES):
            e = band[c]
            assign[c].append((e, [extra_ptr[e]]))
            extra_ptr[e] += 1
    assert all(extra_ptr[e] == FT for e in range(E))
    # emission order: regular groups ascending by nt (small first => fast
    # start) with the extras interleaved mid-stream so their DMA bursts
    # amortize into the big groups' windows (extras are single-chunk and
    # locally DMA-bound); the final group is a regular 5-chunk one whose
    # down-proj tail overlaps the output drain
    regs = sorted(range(NCORES), key=lambda g: groups[g][1])
    exts = sorted(range(NCORES, NCORES + NEXTRA), key=lambda g: groups[g][1], reverse=True)
    order = [regs[0], regs[1], exts[0], regs[2], regs[3], exts[1], regs[4], regs[5], exts[2], regs[6], regs[7]]
    groups = [groups[g] for g in order]
    assign = [[asg[g] for g in order] for asg in assign]
    return groups, assign


def _prep_weight_unit(gate_w, up_w, down_w, e, fc):
    """lhsT layouts for one (expert, f-chunk) unit."""
    g16 = gate_w[e][fc * P : (fc + 1) * P].astype(BF16)  # [128, H]
    u16 = up_w[e][fc * P : (fc + 1) * P].astype(BF16)  # [128, H]
    d16 = down_w[e][:, fc * P : (fc + 1) * P].astype(BF16)  # [H, 128]
    # gate/up lhsT: [hp, hc*128+fi] = w[fi, hc*128+hp]
    gt = np.ascontiguousarray(g16.reshape(P, HC, P).transpose(2, 1, 0)).reshape(
        P, HC * P
    )
    ut = np.ascontiguousarray(u16.reshape(P, HC, P).transpose(2, 1, 0)).reshape(
        P, HC * P
    )
    # down lhsT: [fp, ht*128+hi] = d[ht*128+hi, fp]
    dtt = np.ascontiguousarray(d16.T)  # [128, H]
    return gt, ut, dtt


def kernel(x, router_w, gate_w, up_w, down_w):
    from concourse.bass_utils import run_bass_kernel_spmd

    x = np.asarray(x)
    router_w = np.asarray(router_w)
    gate_w = np.asarray(gate_w)
    up_w = np.asarray(up_w)
    down_w = np.asarray(down_w)

    xf = x.reshape(T, H)
    order, probs = _route(xf, router_w)

    # per-expert token lists + combine weights
    idxs, pes = [], []
    for e in range(E):
        sel = (order[:, 0] == e) | (order[:, 1] == e)
        idx = np.nonzero(sel)[0]
        pe = np.where(order[idx, 0] == e, probs[idx, 0], probs[idx, 1])
        idxs.append(idx)
        pes.append(pe.astype(np.float32))
    counts = [len(i) for i in idxs]

    groups, assign = _schedule(counts)
    key = tuple(groups)
    if key not in _nc_cache:
        _nc_cache[key] = _build_nc(groups)
    nc = _nc_cache[key]

    # gathered x per expert in [hp, hc, t] layout (shared across cores)
    xg_cache = {}

    def xg_for(e, nt):
        kk = (e, nt)
        if kk not in xg_cache:
            pad = np.zeros((nt, H), dtype=BF16)
            pad[: counts[e]] = xf[idxs[e]].astype(BF16)
            xg_cache[kk] = np.ascontiguousarray(
                pad.reshape(nt, HC, P).transpose(2, 1, 0)
            )
        return xg_cache[kk]

    in_maps = []
    for c in range(NCORES):
        m = {"wgsrc": np.zeros((1, 1), dtype=BF16)}
        for g, (nf, nt) in enumerate(groups):
            e, fcs = assign[c][g]
            m[f"x{g}"] = xg_for(e, nt)
            gustack = np.empty((nf, P, 2, HC * P), dtype=BF16)
            dstack = np.empty((P, nf, HT * P), dtype=BF16)
            for i, fc in enumerate(fcs):
                gt, ut, dtt = _prep_weight_unit(gate_w, up_w, down_w, e, fc)
                gustack[i, :, 0] = gt
                gustack[i, :, 1] = ut
                dstack[:, i] = dtt
            m[f"g{g}"] = gustack
            m[f"d{g}"] = dstack
        in_maps.append(m)

    res = run_bass_kernel_spmd(
        nc, in_maps, core_ids=list(range(NCORES)), trace=TRACE
    )
    global LAST_RESULT
    LAST_RESULT = res

    # host combine: sum bf16 partials per expert, apply probs, scatter-add
    y_acc = [np.zeros((H, counts[e]), dtype=np.float32) for e in range(E)]
    for c in range(NCORES):
        for g, (nf, nt) in enumerate(groups):
            e, _ = assign[c][g]
            part = np.asarray(res.results[c][f"y{g}"])  # [P, HT, nt] bf16
            part = part.transpose(1, 0, 2).reshape(H, nt)
            y_acc[e] += part[:, : counts[e]].astype(np.float32)
    out_flat = np.zeros((T, H), dtype=np.float32)
    for e in range(E):
        out_flat[idxs[e]] += pes[e][:, None] * y_acc[e].T
    return out_flat.reshape(B, S, H)
